# revision 3
# baseline (speedup 1.0000x reference)
"""AttentionAggregator Trainium2 kernel (8-core SPMD, data-parallel over nodes).

Math (per node b with neighbors n):
  x_att   = lrelu_.01(x @ W_att);  neib_att = lrelu_.01(neibs @ W_att)
  e[b,n]  = lrelu_.2(x_att[b]@a_x + neib_att[b,n]@a_n)
  att     = softmax_n(e)
  agg[b]  = sum_n att[b,n] * neibs[b,n]
  out     = relu([x@W_fcx, agg@W_fcn])

Key transforms (host-side, exact):
  a_h*lrelu(z_h) summed over h is rewritten as
     sum_{seg1} relu(x . col) - sum_{seg2} relu(x . col)
  over 258 precomputed columns:
     seg1 = [.99*|a_h|*w_h : a_h>=0] + [+.01*(W@a)]
     seg2 = [.99*|a_h|*w_h : a_h<0 ] + [-.01*(W@a)]
  using lrelu(u) = .01u + .99 relu(u), a*lrelu(z)=sign(a)*lrelu(|a|z),
  k*relu(u)=relu(k*u) for k>0, and u = relu(u) - relu(-u).

On-chip per 128-node block: per-tile PE transpose of neibs (fp32, exact),
f32r scores matmul (TF32-class, logits only), relu+accumulate drains split
across ACT/DVE, softmax in a transposed [T,128] layout, attention applied
via per-tile [128,4] block-mask matmuls accumulating agg^T in PSUM (fp32),
then exact fp32 output matmuls.
"""
import warnings
warnings.filterwarnings("ignore")
import numpy as np
from contextlib import ExitStack

import concourse.bass as bass
import concourse.tile as tile
from concourse import bacc, mybir, masks
from concourse.bass_utils import run_bass_kernel_spmd

F32 = mybir.dt.float32
F32R = mybir.dt.float32r
AF = mybir.ActivationFunctionType
ALU = mybir.AluOpType
AX = mybir.AxisListType

N_CORES = 8
B_FULL, NB, D, H, O = 20000, 32, 128, 256, 128
HW6 = 2 * H // 2 + 2  # 258 score columns


def _score_weights(W_att: np.ndarray, a_half: np.ndarray):
    """Build the 258-column relu-pair score weight matrix. Returns (W6, split)."""
    pos = np.where(a_half >= 0)[0]
    neg = np.where(a_half < 0)[0]
    Wabs = W_att * np.abs(a_half)[None, :]
    w_d = (W_att @ a_half).astype(np.float64)
    seg1 = np.concatenate([0.99 * Wabs[:, pos], 0.01 * w_d[:, None]], axis=1)
    seg2 = np.concatenate([0.99 * Wabs[:, neg], -0.01 * w_d[:, None]], axis=1)
    W6 = np.concatenate([seg1, seg2], axis=1).astype(np.float32)
    return W6, seg1.shape[1]


def _blocks(bc):
    out = []
    o = 0
    while o < bc:
        f = min(128, bc - o)
        assert f * NB % 128 == 0
        out.append((o, f))
        o += f
    return out


_PROG_CACHE = {}

# Test-harness knobs (ignored by the grading harness, which calls kernel()
# directly): set TRACE_OPTS["trace"]=True to capture an NTFF profile; the
# BassKernelResults of the last run lands in LAST_RESULT[0].
TRACE_OPTS = {}
LAST_RESULT = [None]


def _build_program(bc, split_n, split_x, n_cores=N_CORES, relu_blk=8):
    """Build + compile the SPMD program for bc nodes per core."""
    key = (bc, split_n, split_x, n_cores, relu_blk)
    if key in _PROG_CACHE:
        return _PROG_CACHE[key]

    nc = bacc.Bacc("TRN2", target_bir_lowering=False, debug=False,
                   num_devices=n_cores)

    x_d = nc.dram_tensor("x", [bc, D], F32R, kind="ExternalInput").ap()
    ne_d = nc.dram_tensor("ne", [bc * NB, D], F32R, kind="ExternalInput").ap()
    w6n_d = nc.dram_tensor("w6n", [D, HW6], F32, kind="ExternalInput").ap()
    w6x_d = nc.dram_tensor("w6x", [D, HW6], F32, kind="ExternalInput").ap()
    wfcx_d = nc.dram_tensor("wfcx", [D, O], F32, kind="ExternalInput").ap()
    wfcn_d = nc.dram_tensor("wfcn", [D, O], F32, kind="ExternalInput").ap()
    mask_d = nc.dram_tensor("mask", [128, 4], F32, kind="ExternalInput").ap()
    mask4_d = nc.dram_tensor("mask4", [128, 4], F32, kind="ExternalInput").ap()
    psel_d = nc.dram_tensor("psel", [128, 32], F32, kind="ExternalInput").ap()
    cful_d = nc.dram_tensor("cful", [128, HW6], F32, kind="ExternalInput").ap()
    out_d = nc.dram_tensor("out", [bc, 2 * O], F32, kind="ExternalOutput").ap()

    with tile.TileContext(nc) as tc, ExitStack() as ctx:
        consts = ctx.enter_context(tc.tile_pool(name="consts", bufs=1))
        nepool = ctx.enter_context(tc.tile_pool(name="ne", bufs=4))
        ntpool = ctx.enter_context(tc.tile_pool(name="nt", bufs=3))
        sc1 = ctx.enter_context(tc.tile_pool(name="scr_act", bufs=4))
        sc2 = ctx.enter_context(tc.tile_pool(name="scr_dve", bufs=4))
        blkpool = ctx.enter_context(tc.tile_pool(name="blk", bufs=2))
        ps_sc = ctx.enter_context(tc.tile_pool(name="ps_sc", bufs=3, space="PSUM"))
        ps_nt = ctx.enter_context(tc.tile_pool(name="ps_nt", bufs=2, space="PSUM"))
        ps_agg = ctx.enter_context(tc.tile_pool(name="ps_agg", bufs=1, space="PSUM"))
        ps_misc = ctx.enter_context(tc.tile_pool(name="ps_misc", bufs=2, space="PSUM"))

        ident = consts.tile([128, 128], F32)
        masks.make_identity(nc, ident[:])
        w6n32 = consts.tile([D, HW6], F32)
        w6x32 = consts.tile([D, HW6], F32)
        wfcx = consts.tile([D, O], F32)
        wfcn = consts.tile([D, O], F32)
        mask = consts.tile([128, 4], F32)
        mask4 = consts.tile([128, 4], F32)
        psel = consts.tile([128, 32], F32)
        cful = consts.tile([128, HW6], F32)
        for t, d in [(w6n32, w6n_d), (w6x32, w6x_d), (wfcx, wfcx_d),
                     (wfcn, wfcn_d), (mask, mask_d), (mask4, mask4_d),
                     (psel, psel_d), (cful, cful_d)]:
            nc.sync.dma_start(t[:], d)
        w6n = consts.tile([D, HW6], F32R)
        w6x = consts.tile([D, HW6], F32R)
        identr = consts.tile([128, 128], F32R)
        wfcx_r = consts.tile([D, O], F32R)
        wfcn_r = consts.tile([D, O], F32R)
        psel_r = consts.tile([128, 32], F32R)
        nc.vector.tensor_copy(w6n[:], w6n32[:])
        nc.vector.tensor_copy(w6x[:], w6x32[:])
        nc.vector.tensor_copy(identr[:], ident[:])
        nc.vector.tensor_copy(wfcx_r[:], wfcx[:])
        nc.vector.tensor_copy(wfcn_r[:], wfcn[:])
        nc.vector.tensor_copy(psel_r[:], psel[:])

        def phase1(boff, F):
            T = F * NB // 128  # score tiles in this block
            rbase = boff * NB

            ne_buf = nepool.tile([128, 32 * D], F32R, tag="ne")
            ne_v = ne_buf[:].rearrange("p (t d) -> p t d", d=D)
            nc.sync.dma_start(
                ne_v[:, :T, :],
                ne_d[rbase: rbase + 128 * T, :].rearrange(
                    "(t p) d -> p t d", p=128))

            # ---- x side
            x_sb = blkpool.tile([128, D], F32R, tag="x")
            nc.sync.dma_start(x_sb[:F, :], x_d[boff:boff + F, :])
            xt_ps = ps_misc.tile([128, 258], F32R, tag="misc")
            nc.tensor.transpose(xt_ps[:, :F], x_sb[:F, :], identr[:F, :F])
            xtr = blkpool.tile([D, 128], F32R, tag="xtr")
            nc.vector.tensor_copy(xtr[:, :F], xt_ps[:, :F])
            xs_ps = ps_misc.tile([128, 258], F32, tag="misc")
            nc.tensor.matmul(xs_ps[:F, :], xtr[:, :F], w6x[:], start=True, stop=True)
            sxacc = blkpool.tile([128, 2], F32, tag="sxacc")
            xscr = sc1.tile([128, HW6], F32, tag="scr_a")
            nc.scalar.activation(xscr[:F, :split_x], xs_ps[:F, :split_x], AF.Relu,
                                 accum_out=sxacc[:F, 0:1])
            nc.scalar.activation(xscr[:F, split_x:HW6], xs_ps[:F, split_x:HW6],
                                 AF.Relu, accum_out=sxacc[:F, 1:2])
            sx = blkpool.tile([128, 1], F32, tag="sx")
            nc.vector.tensor_tensor(sx[:F, :], sxacc[:F, 0:1], sxacc[:F, 1:2],
                                    op=ALU.subtract)
            sx4 = blkpool.tile([128, 4], F32, tag="sx4")
            nc.vector.tensor_scalar(sx4[:F, :], mask4[:F, :], sx[:F, 0:1], None,
                                    op0=ALU.mult)
            sxg_ps = ps_misc.tile([128, 258], F32, tag="misc")
            nc.tensor.matmul(sxg_ps[:T, 0:4], psel[:F, :T], sx4[:F, :],
                             start=True, stop=True)
            sxg = blkpool.tile([32, 4], F32, tag="sxg")
            nc.vector.tensor_copy(sxg[:T, :], sxg_ps[:T, 0:4])

            # ---- per-tile: transpose, scores, relu+accum drains
            spos = blkpool.tile([128, 32], F32, tag="spos")
            sneg = blkpool.tile([128, 32], F32, tag="sneg")
            nc.gpsimd.memset(sneg[:, :T], 0.0)

            def emit_scores(t0, npair, nt_sb):
                for k in range(npair):
                    t = t0 + k
                    s_ps = ps_sc.tile([128, HW6], F32, tag="sc")
                    nc.tensor.matmul(s_ps[:], nt_sb[:, 128 * k:128 * (k + 1)],
                                     w6n[:], start=True, stop=True)
                    if t % 10 < 2:
                        scr = sc1.tile([128, HW6], F32, tag="scr_a")
                        nc.scalar.activation(scr[:, :split_n], s_ps[:, :split_n],
                                             AF.Relu, accum_out=spos[:, t:t + 1])
                        nc.scalar.activation(scr[:, split_n:HW6],
                                             s_ps[:, split_n:HW6], AF.Relu,
                                             accum_out=sneg[:, t:t + 1])
                    else:
                        scr = sc2.tile([128, HW6], F32, tag="scr_d")
                        nc.vector.scalar_tensor_tensor(
                            scr[:], s_ps[:], 0.0, cful[:],
                            op0=ALU.max, op1=ALU.mult,
                            accum_out=spos[:, t:t + 1])

            lags = []
            for t0 in range(0, T, 2):
                npair = min(2, T - t0)
                nt_ps = ps_nt.tile([128, 256], F32R, tag="nt")
                for k in range(npair):
                    t = t0 + k
                    nc.tensor.transpose(nt_ps[:, 128 * k:128 * (k + 1)],
                                        ne_v[:, t, :], identr[:])
                nt_sb = ntpool.tile([128, 256], F32R, tag="nt")
                if (t0 // 2) % 4 == 3:
                    nc.vector.tensor_copy(nt_sb[:, :128 * npair],
                                          nt_ps[:, :128 * npair])
                else:
                    nc.scalar.copy(nt_sb[:, :128 * npair], nt_ps[:, :128 * npair])
                lags.append((t0, npair, nt_sb))
                if len(lags) > 1:
                    emit_scores(*lags.pop(0))
            for l in lags:
                emit_scores(*l)

            return dict(ne_v=ne_v, xtr=xtr, T=T, F=F, boff=boff,
                        spos=spos, sneg=sneg, sxg=sxg)

        def phase1b(st):
            T, F = st["T"], st["F"]
            spos, sneg, sxg = st["spos"], st["sneg"], st["sxg"]
            # ---- softmax over neighbors in [T, 128] layout
            s_col = blkpool.tile([128, 32], F32, tag="s_col")
            nc.vector.tensor_tensor(s_col[:, :T], spos[:, :T], sneg[:, :T],
                                    op=ALU.subtract)
            snt_ps = ps_misc.tile([128, 258], F32, tag="misc")
            nc.tensor.transpose(snt_ps[:T, :128], s_col[:, :T], ident[:])
            z = blkpool.tile([32, 128], F32, tag="z")
            nc.vector.tensor_tensor(
                z[:T, :].rearrange("t (j n) -> t j n", n=32),
                snt_ps[:T, :128].rearrange("t (j n) -> t j n", n=32),
                sxg[:T, :].unsqueeze(2).broadcast_to([T, 4, 32]),
                op=ALU.add)
            zl = blkpool.tile([32, 128], F32, tag="zl")
            nc.vector.scalar_tensor_tensor(zl[:T, :], z[:T, :], 0.2, z[:T, :],
                                           op0=ALU.mult, op1=ALU.max)
            ex = blkpool.tile([32, 128], F32, tag="ex")
            nc.scalar.activation(ex[:T, :], zl[:T, :], AF.Exp)
            sums = blkpool.tile([32, 4], F32, tag="sums")
            nc.vector.tensor_reduce(
                sums[:T, :], ex[:T, :].rearrange("t (j n) -> t j n", n=32),
                axis=AX.X, op=ALU.add)
            rec = blkpool.tile([32, 4], F32, tag="rec")
            nc.vector.reciprocal(rec[:T, :], sums[:T, :])
            att = blkpool.tile([32, 128], F32, tag="att")
            nc.vector.tensor_tensor(
                att[:T, :].rearrange("t (j n) -> t j n", n=32),
                ex[:T, :].rearrange("t (j n) -> t j n", n=32),
                rec[:T, :].unsqueeze(2).broadcast_to([T, 4, 32]),
                op=ALU.mult)
            att_ps = ps_misc.tile([128, 258], F32, tag="misc")
            nc.tensor.transpose(att_ps[:, :T], att[:T, :], ident[:T, :T])
            a_all = blkpool.tile([128, 128], F32R, tag="a_all")
            nc.vector.tensor_tensor(
                a_all[:].rearrange("p (t j) -> p t j", j=4)[:, :T, :],
                mask[:].unsqueeze(1).broadcast_to([128, T, 4]),
                att_ps[:, :T].unsqueeze(2).broadcast_to([128, T, 4]),
                op=ALU.mult)
            st["a_all"] = a_all

        def phase2(st):
            ne_v, a_all, xtr = st["ne_v"], st["a_all"], st["xtr"]
            T, F, boff = st["T"], st["F"], st["boff"]
            agg_ps = ps_agg.tile([128, 128], F32, tag="agg")
            a_v = a_all[:].rearrange("p (t j) -> p t j", j=4)
            for t in range(T):
                nc.tensor.matmul(agg_ps[:, 4 * t:4 * (t + 1)], ne_v[:, t, :],
                                 a_v[:, t, :], start=True, stop=True)
            aggt = blkpool.tile([D, 128], F32R, tag="aggt")
            nc.vector.tensor_copy(aggt[:, :F], agg_ps[:, :F])

            fc_ps = ps_misc.tile([128, 258], F32, tag="misc")
            nc.tensor.matmul(fc_ps[:F, 0:O], xtr[:, :F], wfcx_r[:],
                             start=True, stop=True)
            nc.tensor.matmul(fc_ps[:F, O:2 * O], aggt[:, :F], wfcn_r[:],
                             start=True, stop=True)
            out_sb = blkpool.tile([128, 2 * O], F32, tag="out")
            nc.vector.tensor_scalar(out_sb[:F, :], fc_ps[:F, :2 * O], 0.0, None,
                                    op0=ALU.max)
            nc.sync.dma_start(out_d[boff:boff + F, :], out_sb[:F, :])

        prev = None
        for (boff, F) in _blocks(bc):
            st = phase1(boff, F)
            if prev is not None:
                phase2(prev)
            phase1b(st)
            prev = st
        phase2(prev)

    nc.compile()
    _PROG_CACHE[key] = nc
    return nc


def kernel(x, neibs, W_att, W_fcx, W_fcn, a, n_cores=N_CORES):
    x = np.asarray(x, dtype=np.float32)
    neibs = np.asarray(neibs, dtype=np.float32)
    W_att = np.asarray(W_att, dtype=np.float32)
    W_fcx = np.asarray(W_fcx, dtype=np.float32)
    W_fcn = np.asarray(W_fcn, dtype=np.float32)
    a = np.asarray(a, dtype=np.float32)

    B = x.shape[0]
    bc = B // n_cores
    a_x, a_n = a[:H, 0], a[H:, 0]
    w6x_np, split_x = _score_weights(W_att, a_x)
    w6n_np, split_n = _score_weights(W_att, a_n)
    mask_np = np.equal.outer(np.arange(128) // 32, np.arange(4)).astype(np.float32)
    mask4_np = np.equal.outer(np.arange(128) % 4, np.arange(4)).astype(np.float32)
    psel_np = np.equal.outer(np.arange(128) // 4, np.arange(32)).astype(np.float32)

    nc = _build_program(bc, split_n, split_x, n_cores)

    cvec = np.concatenate([np.ones(split_n), -np.ones(HW6 - split_n)]).astype(np.float32)
    cful_np = np.repeat(cvec[None, :], 128, axis=0)
    shared = {"w6n": w6n_np, "w6x": w6x_np, "wfcx": W_fcx, "wfcn": W_fcn,
              "mask": mask_np, "mask4": mask4_np, "psel": psel_np, "cful": cful_np}
    in_maps = []
    for c in range(n_cores):
        in_maps.append({
            "x": x[c * bc:(c + 1) * bc],
            "ne": neibs[c * bc * NB:(c + 1) * bc * NB],
            **shared,
        })
    res = run_bass_kernel_spmd(nc, in_maps, core_ids=list(range(n_cores)),
                               **TRACE_OPTS)
    LAST_RESULT[0] = res
    return np.concatenate([res.results[c]["out"] for c in range(n_cores)], axis=0)



# revision 18
# speedup vs baseline: 1.4743x; 1.4743x over previous
"""AttentionAggregator Trainium2 kernel (8-core SPMD, data-parallel over nodes).

Math (per node b with neighbors n):
  x_att   = lrelu_.01(x @ W_att);  neib_att = lrelu_.01(neibs @ W_att)
  e[b,n]  = lrelu_.2(x_att[b]@a_x + neib_att[b,n]@a_n)
  att     = softmax_n(e)
  agg[b]  = sum_n att[b,n] * neibs[b,n]
  out     = relu([x@W_fcx, agg@W_fcn])

v3 design (vs the PE-transpose baseline):
  - Host pre-builds TWO fp16 layouts of neibs per core: neT (transposed
    [D, rows] — feeds score matmuls directly, no PE transposes) and neN
    (rows-grouped-by-128 natural [p, (tile, d)] — feeds the attention-apply
    matmuls).  Both DMA at 8KB/partition-line per block.
  - x is pre-transposed on host (xT [D, B]) and persists in SBUF: serves as
    lhsT for both the x-score matmul and the x@W_fcx output matmul.
  - Scores: 258-col relu-pair decomposition of a.lrelu(W z) (exact), drained
    by DVE scalar_tensor_tensor(max0, *cful, accum) / ACT-relu+GPSIMD-reduce.
  - Softmax normalization without cross-partition shuffles: per-node sums,
    reciprocal-broadcast and x-score broadcast are done with tiny constant
    selector matmuls on the PE (psel4/bsel4/bselx).
"""
import warnings
warnings.filterwarnings("ignore")
import numpy as np
from contextlib import ExitStack

import concourse.bass as bass
import concourse.tile as tile
from concourse import bacc, mybir
from concourse.bass_utils import run_bass_kernel_spmd

F32 = mybir.dt.float32
F16 = mybir.dt.float16
AF = mybir.ActivationFunctionType
ALU = mybir.AluOpType
AX = mybir.AxisListType

N_CORES = 8
B_FULL, NB, D, H, O = 20000, 32, 128, 256, 128
HW6 = 2 * H // 2 + 2  # 258 score columns

# Test-harness knobs (ignored by the grading harness, which calls kernel()
# directly): set TRACE_OPTS["trace"]=True to capture an NTFF profile; the
# BassKernelResults of the last run lands in LAST_RESULT[0].
TRACE_OPTS = {}
LAST_RESULT = [None]

# Drain scheduling: score PSUM tiles are consumed either directly on DVE
# (scalar_tensor_tensor max0*cful+accum, ~394ns) or via an ACT relu pass over
# a 4-tile PSUM quad into SBUF fp16, after which the per-tile signed sum runs
# as 4x-mode tensor_scalar ops on DVE — with GPSIMD pre-applying the +-1
# column signs for half of each quad so those tiles need only ONE DVE op.
N_QUADS = 7  # ACT-relu quads per 32-tile block (4 tiles each); rest DVE-direct


def _score_weights(W_att: np.ndarray, a_half: np.ndarray):
    """Build the 258-column relu-pair score weight matrix. Returns (W6, split)."""
    pos = np.where(a_half >= 0)[0]
    neg = np.where(a_half < 0)[0]
    Wabs = W_att * np.abs(a_half)[None, :]
    w_d = (W_att @ a_half).astype(np.float64)
    seg1 = np.concatenate([0.99 * Wabs[:, pos], 0.01 * w_d[:, None]], axis=1)
    seg2 = np.concatenate([0.99 * Wabs[:, neg], -0.01 * w_d[:, None]], axis=1)
    W6 = np.concatenate([seg1, seg2], axis=1).astype(np.float32)
    return W6, seg1.shape[1]


def _blocks(bc):
    out = []
    o = 0
    while o < bc:
        f = min(128, bc - o)
        assert f * NB % 128 == 0
        out.append((o, f))
        o += f
    return out


_PROG_CACHE = {}


def _build_program(bc, split_n, split_x, n_cores=N_CORES):
    key = (bc, split_n, split_x, n_cores, N_QUADS)
    if key in _PROG_CACHE:
        return _PROG_CACHE[key]

    nc = bacc.Bacc("TRN2", target_bir_lowering=False, debug=False,
                   num_devices=n_cores)

    R = bc * NB  # neighbor rows per core
    neT_d = nc.dram_tensor("neT", [D, R], F16, kind="ExternalInput").ap()
    neN_d = nc.dram_tensor("neN", [128, R], F16, kind="ExternalInput").ap()
    xT_d = nc.dram_tensor("xT", [D, bc], F16, kind="ExternalInput").ap()
    w6n_d = nc.dram_tensor("w6n", [D, HW6], F16, kind="ExternalInput").ap()
    w6x_d = nc.dram_tensor("w6x", [D, HW6], F16, kind="ExternalInput").ap()
    cfn_d = nc.dram_tensor("cfn", [128, HW6], F16, kind="ExternalInput").ap()
    cfn2_d = nc.dram_tensor("cfn2", [128, 2 * HW6], F16, kind="ExternalInput").ap()
    cfx_d = nc.dram_tensor("cfx", [128, HW6], F16, kind="ExternalInput").ap()
    wfcx_d = nc.dram_tensor("wfcx", [D, O], F16, kind="ExternalInput").ap()
    wfcn_d = nc.dram_tensor("wfcn", [D, O], F16, kind="ExternalInput").ap()
    psel4_d = nc.dram_tensor("psel4", [128, 4], F32, kind="ExternalInput").ap()
    bsel4_d = nc.dram_tensor("bsel4", [4, 128], F32, kind="ExternalInput").ap()
    bselx_d = nc.dram_tensor("bselx", [128, 128], F32, kind="ExternalInput").ap()
    selm_d = nc.dram_tensor("selm", [128, NB], F32, kind="ExternalInput").ap()
    mask4_d = nc.dram_tensor("mask4", [128, 4], F16, kind="ExternalInput").ap()
    out_d = nc.dram_tensor("out", [bc, 2 * O], F32, kind="ExternalOutput").ap()

    with tile.TileContext(nc) as tc, ExitStack() as ctx:
        consts = ctx.enter_context(tc.tile_pool(name="consts", bufs=1))
        netp = ctx.enter_context(tc.tile_pool(name="netp", bufs=2))
        nenp = ctx.enter_context(tc.tile_pool(name="nenp", bufs=2))
        scrp = ctx.enter_context(tc.tile_pool(name="scrp", bufs=3))
        smallp = ctx.enter_context(tc.tile_pool(name="smallp", bufs=2))
        awp = ctx.enter_context(tc.tile_pool(name="awp", bufs=2))
        outp = ctx.enter_context(tc.tile_pool(name="outp", bufs=2))
        ps_sc = ctx.enter_context(tc.tile_pool(name="ps_sc", bufs=1, space="PSUM"))
        ps_quad = ctx.enter_context(tc.tile_pool(name="ps_quad", bufs=1, space="PSUM"))
        ps_misc = ctx.enter_context(tc.tile_pool(name="ps_misc", bufs=1, space="PSUM"))
        ps_agg = ctx.enter_context(tc.tile_pool(name="ps_agg", bufs=1, space="PSUM"))
        ps_fc = ctx.enter_context(tc.tile_pool(name="ps_fc", bufs=1, space="PSUM"))

        w6n = consts.tile([D, HW6], F16)
        w6x = consts.tile([D, HW6], F16)
        cfn = consts.tile([128, HW6], F16)
        cfn2 = consts.tile([128, 2 * HW6], F16)
        cfx = consts.tile([128, HW6], F16)
        wfcx = consts.tile([D, O], F16)
        wfcn = consts.tile([D, O], F16)
        psel4 = consts.tile([128, 4], F32)
        bsel4 = consts.tile([4, 128], F32)
        bselx = consts.tile([128, 128], F32)
        selm = consts.tile([128, NB], F32)
        mask4 = consts.tile([128, 4], F16)
        xt_all = consts.tile([D, bc], F16)
        for t, d in [(w6n, w6n_d), (w6x, w6x_d), (cfn, cfn_d), (cfn2, cfn2_d),
                     (cfx, cfx_d),
                     (wfcx, wfcx_d), (wfcn, wfcn_d), (psel4, psel4_d),
                     (bsel4, bsel4_d), (bselx, bselx_d), (selm, selm_d),
                     (mask4, mask4_d), (xt_all, xT_d)]:
            nc.sync.dma_start(t[:], d)

        def phaseA(boff, F):
            """DMA loads, score matmuls + drains, x-side score."""
            T = F * NB // 128
            rbase = boff * NB

            netb = netp.tile([128, 32 * D], F16, tag="net")
            nc.sync.dma_start(netb[:, :T * D], neT_d[:, rbase:rbase + 128 * T])
            nenb = nenp.tile([128, 32 * D], F16, tag="nen")
            nc.sync.dma_start(nenb[:, :T * D], neN_d[:, rbase:rbase + 128 * T])

            scolP = smallp.tile([128, NB], F32, tag="scolP")
            scolN = smallp.tile([128, NB], F32, tag="scolN")
            nc.gpsimd.memset(scolN[:, :T], 0.0)

            # x-side score (drain on DVE)
            xs_ps = ps_sc.tile([128, HW6], F32, tag="sc")
            nc.tensor.matmul(xs_ps[:F, :], xt_all[:, boff:boff + F], w6x[:],
                             start=True, stop=True)
            xscr = scrp.tile([128, HW6], F16, tag="xscr")
            sx = smallp.tile([128, 1], F32, tag="sx")
            nc.vector.scalar_tensor_tensor(
                xscr[:F, :], xs_ps[:F, :], 0.0, cfx[:F, :],
                op0=ALU.max, op1=ALU.mult, accum_out=sx[:F, :])

            # neighbor score tiles: ACT-relu quads (with GPSIMD sign-apply +
            # DVE 4x-mode sums) and DVE-direct singles
            s1 = split_n
            s2 = HW6 - split_n
            t = 0
            quad_budget = N_QUADS
            while t < T:
                if quad_budget > 0 and t + 4 <= T:
                    quad_budget -= 1
                    qp = ps_quad.tile([128, 2048], F32, tag="qp")
                    qv = qp[:].rearrange("p (g c) -> p g c", c=512)
                    for g in range(4):
                        nc.tensor.matmul(qv[:, g, :HW6],
                                         netb[:, (t + g) * D:(t + g + 1) * D],
                                         w6n[:], start=True, stop=True)
                    scr4 = scrp.tile([128, 4 * HW6], F16, tag="scr4")
                    nc.scalar.activation(
                        scr4[:].rearrange("p (g c) -> p g c", c=HW6),
                        qv[:, :, :HW6], AF.Relu)
                    # tiles t+0,t+1: unsigned two-segment sums on DVE (4x)
                    dmy = scrp.tile([128, HW6], F16, tag="dmy")
                    for g in range(2):
                        base = g * HW6
                        nc.vector.tensor_scalar(
                            dmy[:, :s1], scr4[:, base:base + s1], 0.0, 0.0,
                            op0=ALU.bypass, op1=ALU.add,
                            accum_out=scolP[:, t + g:t + g + 1])
                        nc.vector.tensor_scalar(
                            dmy[:, :s2], scr4[:, base + s1:base + HW6], 0.0,
                            0.0, op0=ALU.bypass, op1=ALU.add,
                            accum_out=scolN[:, t + g:t + g + 1])
                    # tiles t+2,t+3: GPSIMD applies column signs, DVE one sum
                    scr4s = scrp.tile([128, 2 * HW6], F16, tag="scr4s")
                    nc.gpsimd.tensor_tensor(
                        scr4s[:], scr4[:, 2 * HW6:4 * HW6], cfn2[:],
                        op=ALU.mult)
                    for g in range(2):
                        nc.vector.tensor_scalar(
                            dmy[:], scr4s[:, g * HW6:(g + 1) * HW6], 0.0, 0.0,
                            op0=ALU.bypass, op1=ALU.add,
                            accum_out=scolP[:, t + 2 + g:t + 3 + g])
                    t += 4
                else:
                    s_ps = ps_sc.tile([128, HW6], F32, tag="sc")
                    nc.tensor.matmul(s_ps[:], netb[:, t * D:(t + 1) * D],
                                     w6n[:], start=True, stop=True)
                    scr = scrp.tile([128, HW6], F16, tag="scr")
                    nc.vector.scalar_tensor_tensor(
                        scr[:], s_ps[:], 0.0, cfn[:],
                        op0=ALU.max, op1=ALU.mult,
                        accum_out=scolP[:, t:t + 1])
                    t += 1

            return dict(nenb=nenb, T=T, F=F, boff=boff, scolP=scolP,
                        scolN=scolN, sx=sx)

        def phaseB(st):
            """Softmax (unnormalized exp + reciprocal-Z) and attention weights."""
            T, F = st["T"], st["F"]
            scolP, scolN, sx = st["scolP"], st["scolN"], st["sx"]

            # sxs[p,t] = sx[4t + p//32] via selector matmul
            Rm = smallp.tile([128, NB], F32, tag="Rm")
            nc.vector.tensor_tensor(
                Rm[:, :T], sx[:, 0:1].broadcast_to([128, T]), selm[:, :T],
                op=ALU.mult)
            sxs_ps = ps_misc.tile([128, NB], F32, tag="misc")
            nc.tensor.matmul(sxs_ps[:, :T], bselx[:], Rm[:, :T],
                             start=True, stop=True)

            z0 = smallp.tile([128, NB], F32, tag="z0")
            nc.vector.tensor_tensor(z0[:, :T], scolP[:, :T], scolN[:, :T],
                                    op=ALU.subtract)
            z = smallp.tile([128, NB], F32, tag="z")
            nc.vector.tensor_tensor(z[:, :T], z0[:, :T], sxs_ps[:, :T],
                                    op=ALU.add)
            zl = smallp.tile([128, NB], F32, tag="zl")
            nc.vector.scalar_tensor_tensor(zl[:, :T], z[:, :T], 0.2, z[:, :T],
                                           op0=ALU.mult, op1=ALU.max)
            ew = smallp.tile([128, NB], F32, tag="ew")
            nc.scalar.activation(ew[:, :T], zl[:, :T], AF.Exp)

            # Z per node, reciprocal, broadcast back to [p, t]
            zt_ps = ps_misc.tile([128, NB], F32, tag="misc")
            nc.tensor.matmul(zt_ps[:4, :T], psel4[:], ew[:, :T],
                             start=True, stop=True)
            rz4 = smallp.tile([4, NB], F32, tag="rz4")
            nc.vector.reciprocal(rz4[:4, :T], zt_ps[:4, :T])
            rzf_ps = ps_misc.tile([128, NB], F32, tag="misc")
            nc.tensor.matmul(rzf_ps[:, :T], bsel4[:4, :], rz4[:4, :T],
                             start=True, stop=True)
            ewn = smallp.tile([128, NB], F32, tag="ewn")
            nc.vector.tensor_tensor(ewn[:, :T], ew[:, :T], rzf_ps[:, :T],
                                    op=ALU.mult)

            aw = awp.tile([128, 128], F16, tag="aw")
            nc.vector.tensor_tensor(
                aw[:].rearrange("p (t j) -> p t j", j=4)[:, :T, :],
                ewn[:, :T].unsqueeze(2).broadcast_to([128, T, 4]),
                mask4[:].unsqueeze(1).broadcast_to([128, T, 4]),
                op=ALU.mult)
            st["aw"] = aw

        def phaseC(st):
            """Attention apply (agg), output matmuls, relu, store."""
            nenb, aw = st["nenb"], st["aw"]
            T, F, boff = st["T"], st["F"], st["boff"]
            nen_v = nenb[:].rearrange("p (t d) -> p t d", d=D)
            aw_v = aw[:].rearrange("p (t j) -> p t j", j=4)
            agg_ps = ps_agg.tile([128, 128], F32, tag="agg")
            for t in range(T):
                nc.tensor.matmul(agg_ps[:, 4 * t:4 * (t + 1)], nen_v[:, t, :],
                                 aw_v[:, t, :], start=True, stop=True)
            aggt = awp.tile([D, 128], F16, tag="aggt")
            nc.vector.tensor_copy(aggt[:, :F], agg_ps[:, :F])

            fc_ps = ps_fc.tile([128, 2 * O], F32, tag="fc")
            nc.tensor.matmul(fc_ps[:F, 0:O], xt_all[:, boff:boff + F], wfcx[:],
                             start=True, stop=True)
            nc.tensor.matmul(fc_ps[:F, O:2 * O], aggt[:, :F], wfcn[:],
                             start=True, stop=True)
            out_sb = outp.tile([128, 2 * O], F32, tag="out")
            nc.scalar.activation(out_sb[:F, :], fc_ps[:F, :], AF.Relu)
            nc.sync.dma_start(out_d[boff:boff + F, :], out_sb[:F, :])

        prev = None
        for (boff, F) in _blocks(bc):
            st = phaseA(boff, F)
            if prev is not None:
                phaseC(prev)
            phaseB(st)
            prev = st
        phaseC(prev)

    nc.compile()
    _PROG_CACHE[key] = nc
    return nc


def kernel(x, neibs, W_att, W_fcx, W_fcn, a, n_cores=N_CORES):
    x = np.asarray(x, dtype=np.float32)
    neibs = np.asarray(neibs, dtype=np.float32)
    W_att = np.asarray(W_att, dtype=np.float32)
    W_fcx = np.asarray(W_fcx, dtype=np.float32)
    W_fcn = np.asarray(W_fcn, dtype=np.float32)
    a = np.asarray(a, dtype=np.float32)

    B = x.shape[0]
    bc = B // n_cores
    a_x, a_n = a[:H, 0], a[H:, 0]
    w6x_np, split_x = _score_weights(W_att, a_x)
    w6n_np, split_n = _score_weights(W_att, a_n)

    nc = _build_program(bc, split_n, split_x, n_cores)

    def cful(split, rep=1):
        v = np.concatenate([np.ones(split), -np.ones(HW6 - split)])
        v = np.tile(v, rep)
        return np.repeat(v[None, :].astype(np.float16), 128, axis=0)

    p = np.arange(128)
    psel4_np = np.equal.outer(p // 32, np.arange(4)).astype(np.float32)
    bsel4_np = np.equal.outer(np.arange(4), p // 32).astype(np.float32)
    bselx_np = np.equal.outer(p % 4, p // 32).astype(np.float32)
    selm_np = np.equal.outer(p // 4, np.arange(NB)).astype(np.float32)
    mask4_np = np.equal.outer(p // 32, np.arange(4)).astype(np.float16)

    shared = {
        "w6n": w6n_np.astype(np.float16), "w6x": w6x_np.astype(np.float16),
        "cfn": cful(split_n), "cfn2": cful(split_n, rep=2), "cfx": cful(split_x),
        "wfcx": W_fcx.astype(np.float16), "wfcn": W_fcn.astype(np.float16),
        "psel4": psel4_np, "bsel4": bsel4_np, "bselx": bselx_np,
        "selm": selm_np, "mask4": mask4_np,
    }

    rows_c = bc * NB
    tiles_c = rows_c // 128
    in_maps = []
    for c in range(n_cores):
        sl = neibs[c * rows_c:(c + 1) * rows_c]
        neT_np = np.ascontiguousarray(sl.T).astype(np.float16)
        neN_np = np.ascontiguousarray(
            sl.reshape(tiles_c, 128, D).transpose(1, 0, 2).reshape(128, rows_c)
        ).astype(np.float16)
        xT_np = np.ascontiguousarray(x[c * bc:(c + 1) * bc].T).astype(np.float16)
        in_maps.append({
            "neT": neT_np, "neN": neN_np, "xT": xT_np, **shared,
        })
    res = run_bass_kernel_spmd(nc, in_maps, core_ids=list(range(n_cores)),
                               **TRACE_OPTS)
    LAST_RESULT[0] = res
    return np.concatenate([res.results[c]["out"] for c in range(n_cores)], axis=0)


# revision 21
# speedup vs baseline: 1.5635x; 1.0605x over previous
"""AttentionAggregator Trainium2 kernel (8-core SPMD, data-parallel over nodes).

Math (per node b with neighbors n):
  x_att   = lrelu_.01(x @ W_att);  neib_att = lrelu_.01(neibs @ W_att)
  e[b,n]  = lrelu_.2(x_att[b]@a_x + neib_att[b,n]@a_n)
  att     = softmax_n(e)
  agg[b]  = sum_n att[b,n] * neibs[b,n]
  out     = relu([x@W_fcx, agg@W_fcn])

v3 design (vs the PE-transpose baseline):
  - Host pre-builds TWO fp16 layouts of neibs per core: neT (transposed
    [D, rows] — feeds score matmuls directly, no PE transposes) and neN
    (rows-grouped-by-128 natural [p, (tile, d)] — feeds the attention-apply
    matmuls).  Both DMA at 8KB/partition-line per block.
  - x is pre-transposed on host (xT [D, B]) and persists in SBUF: serves as
    lhsT for both the x-score matmul and the x@W_fcx output matmul.
  - Scores: 258-col relu-pair decomposition of a.lrelu(W z) (exact), drained
    by DVE scalar_tensor_tensor(max0, *cful, accum) / ACT-relu+GPSIMD-reduce.
  - Softmax normalization without cross-partition shuffles: per-node sums,
    reciprocal-broadcast and x-score broadcast are done with tiny constant
    selector matmuls on the PE (psel4/bsel4/bselx).
"""
import warnings
warnings.filterwarnings("ignore")
import numpy as np
from contextlib import ExitStack

import concourse.bass as bass
import concourse.tile as tile
from concourse import bacc, mybir
from concourse.bass_utils import run_bass_kernel_spmd

F32 = mybir.dt.float32
F16 = mybir.dt.float16
AF = mybir.ActivationFunctionType
ALU = mybir.AluOpType
AX = mybir.AxisListType

N_CORES = 8
B_FULL, NB, D, H, O = 20000, 32, 128, 256, 128
HW6 = 2 * H // 2 + 2  # 258 score columns

# Test-harness knobs (ignored by the grading harness, which calls kernel()
# directly): set TRACE_OPTS["trace"]=True to capture an NTFF profile; the
# BassKernelResults of the last run lands in LAST_RESULT[0].
TRACE_OPTS = {}
LAST_RESULT = [None]

# Drain scheduling: score PSUM tiles are consumed via an ACT relu pass over a
# 4-tile PSUM quad into SBUF fp16, then per-quad GROUPED tensor_reduce ops on
# DVE ([128,4,seg]->[128,4], one instruction per sign-segment, no accumulator
# read).  Odd tail tiles fall back to a direct DVE scalar_tensor_tensor drain.
N_QUADS = 8  # ACT-relu quads per 32-tile block (4 tiles each); rest DVE-direct


def _score_weights(W_att: np.ndarray, a_half: np.ndarray):
    """Build the 258-column relu-pair score weight matrix. Returns (W6, split)."""
    pos = np.where(a_half >= 0)[0]
    neg = np.where(a_half < 0)[0]
    Wabs = W_att * np.abs(a_half)[None, :]
    w_d = (W_att @ a_half).astype(np.float64)
    seg1 = np.concatenate([0.99 * Wabs[:, pos], 0.01 * w_d[:, None]], axis=1)
    seg2 = np.concatenate([0.99 * Wabs[:, neg], -0.01 * w_d[:, None]], axis=1)
    W6 = np.concatenate([seg1, seg2], axis=1).astype(np.float32)
    return W6, seg1.shape[1]


def _blocks(bc):
    out = []
    o = 0
    while o < bc:
        f = min(128, bc - o)
        assert f * NB % 128 == 0
        out.append((o, f))
        o += f
    return out


_PROG_CACHE = {}


def _build_program(bc, split_n, split_x, n_cores=N_CORES):
    key = (bc, split_n, split_x, n_cores, N_QUADS)
    if key in _PROG_CACHE:
        return _PROG_CACHE[key]

    nc = bacc.Bacc("TRN2", target_bir_lowering=False, debug=False,
                   num_devices=n_cores)

    R = bc * NB  # neighbor rows per core
    neT_d = nc.dram_tensor("neT", [D, R], F16, kind="ExternalInput").ap()
    neN_d = nc.dram_tensor("neN", [128, R], F16, kind="ExternalInput").ap()
    xT_d = nc.dram_tensor("xT", [D, bc], F16, kind="ExternalInput").ap()
    w6n_d = nc.dram_tensor("w6n", [D, HW6], F16, kind="ExternalInput").ap()
    w6x_d = nc.dram_tensor("w6x", [D, HW6], F16, kind="ExternalInput").ap()
    cfn_d = nc.dram_tensor("cfn", [128, HW6], F16, kind="ExternalInput").ap()
    cfx_d = nc.dram_tensor("cfx", [128, HW6], F16, kind="ExternalInput").ap()
    wfcx_d = nc.dram_tensor("wfcx", [D, O], F16, kind="ExternalInput").ap()
    wfcn_d = nc.dram_tensor("wfcn", [D, O], F16, kind="ExternalInput").ap()
    psel4_d = nc.dram_tensor("psel4", [128, 4], F32, kind="ExternalInput").ap()
    bsel4_d = nc.dram_tensor("bsel4", [4, 128], F32, kind="ExternalInput").ap()
    bselx_d = nc.dram_tensor("bselx", [128, 128], F32, kind="ExternalInput").ap()
    selm_d = nc.dram_tensor("selm", [128, NB], F32, kind="ExternalInput").ap()
    mask4_d = nc.dram_tensor("mask4", [128, 4], F16, kind="ExternalInput").ap()
    out_d = nc.dram_tensor("out", [bc, 2 * O], F32, kind="ExternalOutput").ap()

    with tile.TileContext(nc) as tc, ExitStack() as ctx:
        consts = ctx.enter_context(tc.tile_pool(name="consts", bufs=1))
        netp = ctx.enter_context(tc.tile_pool(name="netp", bufs=2))
        nenp = ctx.enter_context(tc.tile_pool(name="nenp", bufs=2))
        scrp = ctx.enter_context(tc.tile_pool(name="scrp", bufs=3))
        smallp = ctx.enter_context(tc.tile_pool(name="smallp", bufs=2))
        awp = ctx.enter_context(tc.tile_pool(name="awp", bufs=2))
        outp = ctx.enter_context(tc.tile_pool(name="outp", bufs=2))
        ps_sc = ctx.enter_context(tc.tile_pool(name="ps_sc", bufs=1, space="PSUM"))
        ps_quad = ctx.enter_context(tc.tile_pool(name="ps_quad", bufs=1, space="PSUM"))
        ps_misc = ctx.enter_context(tc.tile_pool(name="ps_misc", bufs=1, space="PSUM"))
        ps_agg = ctx.enter_context(tc.tile_pool(name="ps_agg", bufs=1, space="PSUM"))
        ps_fc = ctx.enter_context(tc.tile_pool(name="ps_fc", bufs=1, space="PSUM"))

        w6n = consts.tile([D, HW6], F16)
        w6x = consts.tile([D, HW6], F16)
        cfn = consts.tile([128, HW6], F16)
        cfx = consts.tile([128, HW6], F16)
        wfcx = consts.tile([D, O], F16)
        wfcn = consts.tile([D, O], F16)
        psel4 = consts.tile([128, 4], F32)
        bsel4 = consts.tile([4, 128], F32)
        bselx = consts.tile([128, 128], F32)
        selm = consts.tile([128, NB], F32)
        mask4 = consts.tile([128, 4], F16)
        xt_all = consts.tile([D, bc], F16)
        for t, d in [(w6n, w6n_d), (w6x, w6x_d), (cfn, cfn_d), (cfx, cfx_d),
                     (wfcx, wfcx_d), (wfcn, wfcn_d), (psel4, psel4_d),
                     (bsel4, bsel4_d), (bselx, bselx_d), (selm, selm_d),
                     (mask4, mask4_d), (xt_all, xT_d)]:
            nc.sync.dma_start(t[:], d)

        def phaseA(boff, F):
            """DMA loads, score matmuls + drains, x-side score."""
            T = F * NB // 128
            rbase = boff * NB

            netb = netp.tile([128, 32 * D], F16, tag="net")
            nc.sync.dma_start(netb[:, :T * D], neT_d[:, rbase:rbase + 128 * T])
            nenb = nenp.tile([128, 32 * D], F16, tag="nen")
            nc.sync.dma_start(nenb[:, :T * D], neN_d[:, rbase:rbase + 128 * T])

            scolP = smallp.tile([128, NB], F32, tag="scolP")
            scolN = smallp.tile([128, NB], F32, tag="scolN")
            nc.gpsimd.memset(scolN[:, :T], 0.0)

            # x-side score (drain on DVE)
            xs_ps = ps_sc.tile([128, HW6], F32, tag="sc")
            nc.tensor.matmul(xs_ps[:F, :], xt_all[:, boff:boff + F], w6x[:],
                             start=True, stop=True)
            xscr = scrp.tile([128, HW6], F16, tag="xscr")
            sx = smallp.tile([128, 1], F32, tag="sx")
            nc.vector.scalar_tensor_tensor(
                xscr[:F, :], xs_ps[:F, :], 0.0, cfx[:F, :],
                op0=ALU.max, op1=ALU.mult, accum_out=sx[:F, :])

            # neighbor score tiles: ACT-relu quads + grouped DVE tensor_reduce
            # per sign-segment; odd tail tiles drain directly on DVE
            s1 = split_n
            t = 0
            quad_budget = N_QUADS
            while t < T:
                if quad_budget > 0 and t + 4 <= T:
                    quad_budget -= 1
                    qp = ps_quad.tile([128, 2048], F32, tag="qp")
                    qv = qp[:].rearrange("p (g c) -> p g c", c=512)
                    for g in range(4):
                        nc.tensor.matmul(qv[:, g, :HW6],
                                         netb[:, (t + g) * D:(t + g + 1) * D],
                                         w6n[:], start=True, stop=True)
                    scr4 = scrp.tile([128, 4 * HW6], F16, tag="scr4")
                    scr4v = scr4[:].rearrange("p (g c) -> p g c", c=HW6)
                    nc.scalar.activation(scr4v, qv[:, :, :HW6], AF.Relu)
                    nc.vector.tensor_reduce(
                        scolP[:, t:t + 4], scr4v[:, :, :s1],
                        axis=AX.X, op=ALU.add)
                    nc.vector.tensor_reduce(
                        scolN[:, t:t + 4], scr4v[:, :, s1:HW6],
                        axis=AX.X, op=ALU.add)
                    t += 4
                else:
                    s_ps = ps_sc.tile([128, HW6], F32, tag="sc")
                    nc.tensor.matmul(s_ps[:], netb[:, t * D:(t + 1) * D],
                                     w6n[:], start=True, stop=True)
                    scr = scrp.tile([128, HW6], F16, tag="scr")
                    nc.vector.scalar_tensor_tensor(
                        scr[:], s_ps[:], 0.0, cfn[:],
                        op0=ALU.max, op1=ALU.mult,
                        accum_out=scolP[:, t:t + 1])
                    t += 1

            return dict(nenb=nenb, T=T, F=F, boff=boff, scolP=scolP,
                        scolN=scolN, sx=sx)

        def phaseB(st):
            """Softmax (unnormalized exp + reciprocal-Z) and attention weights."""
            T, F = st["T"], st["F"]
            scolP, scolN, sx = st["scolP"], st["scolN"], st["sx"]

            # sxs[p,t] = sx[4t + p//32] via selector matmul
            Rm = smallp.tile([128, NB], F32, tag="Rm")
            nc.gpsimd.tensor_tensor(
                Rm[:, :T], sx[:, 0:1].broadcast_to([128, T]), selm[:, :T],
                op=ALU.mult)
            sxs_ps = ps_misc.tile([128, NB], F32, tag="misc")
            nc.tensor.matmul(sxs_ps[:, :T], bselx[:], Rm[:, :T],
                             start=True, stop=True)

            z0 = smallp.tile([128, NB], F32, tag="z0")
            nc.gpsimd.tensor_tensor(z0[:, :T], scolP[:, :T], scolN[:, :T],
                                    op=ALU.subtract)
            z = smallp.tile([128, NB], F32, tag="z")
            nc.vector.tensor_tensor(z[:, :T], z0[:, :T], sxs_ps[:, :T],
                                    op=ALU.add)
            zl = smallp.tile([128, NB], F32, tag="zl")
            nc.vector.scalar_tensor_tensor(zl[:, :T], z[:, :T], 0.2, z[:, :T],
                                           op0=ALU.mult, op1=ALU.max)
            ew = smallp.tile([128, NB], F32, tag="ew")
            nc.scalar.activation(ew[:, :T], zl[:, :T], AF.Exp)

            # Z per node, reciprocal, broadcast back to [p, t]
            zt_ps = ps_misc.tile([128, NB], F32, tag="misc")
            nc.tensor.matmul(zt_ps[:4, :T], psel4[:], ew[:, :T],
                             start=True, stop=True)
            rz4 = smallp.tile([4, NB], F32, tag="rz4")
            nc.vector.reciprocal(rz4[:4, :T], zt_ps[:4, :T])
            rzf_ps = ps_misc.tile([128, NB], F32, tag="misc")
            nc.tensor.matmul(rzf_ps[:, :T], bsel4[:4, :], rz4[:4, :T],
                             start=True, stop=True)
            ewn = smallp.tile([128, NB], F32, tag="ewn")
            nc.vector.tensor_tensor(ewn[:, :T], ew[:, :T], rzf_ps[:, :T],
                                    op=ALU.mult)

            aw = awp.tile([128, 128], F16, tag="aw")
            nc.gpsimd.tensor_tensor(
                aw[:].rearrange("p (t j) -> p t j", j=4)[:, :T, :],
                ewn[:, :T].unsqueeze(2).broadcast_to([128, T, 4]),
                mask4[:].unsqueeze(1).broadcast_to([128, T, 4]),
                op=ALU.mult)
            st["aw"] = aw

        def phaseC(st):
            """Attention apply (agg), output matmuls, relu, store."""
            nenb, aw = st["nenb"], st["aw"]
            T, F, boff = st["T"], st["F"], st["boff"]
            nen_v = nenb[:].rearrange("p (t d) -> p t d", d=D)
            aw_v = aw[:].rearrange("p (t j) -> p t j", j=4)
            agg_ps = ps_agg.tile([128, 128], F32, tag="agg")
            for t in range(T):
                nc.tensor.matmul(agg_ps[:, 4 * t:4 * (t + 1)], nen_v[:, t, :],
                                 aw_v[:, t, :], start=True, stop=True)
            aggt = awp.tile([D, 128], F16, tag="aggt")
            nc.vector.tensor_copy(aggt[:, :F], agg_ps[:, :F])

            fc_ps = ps_fc.tile([128, 2 * O], F32, tag="fc")
            nc.tensor.matmul(fc_ps[:F, 0:O], xt_all[:, boff:boff + F], wfcx[:],
                             start=True, stop=True)
            nc.tensor.matmul(fc_ps[:F, O:2 * O], aggt[:, :F], wfcn[:],
                             start=True, stop=True)
            out_sb = outp.tile([128, 2 * O], F32, tag="out")
            nc.scalar.activation(out_sb[:F, :], fc_ps[:F, :], AF.Relu)
            nc.sync.dma_start(out_d[boff:boff + F, :], out_sb[:F, :])

        prev = None
        for (boff, F) in _blocks(bc):
            st = phaseA(boff, F)
            if prev is not None:
                phaseC(prev)
            phaseB(st)
            prev = st
        phaseC(prev)

    nc.compile()
    _PROG_CACHE[key] = nc
    return nc


def kernel(x, neibs, W_att, W_fcx, W_fcn, a, n_cores=N_CORES):
    x = np.asarray(x, dtype=np.float32)
    neibs = np.asarray(neibs, dtype=np.float32)
    W_att = np.asarray(W_att, dtype=np.float32)
    W_fcx = np.asarray(W_fcx, dtype=np.float32)
    W_fcn = np.asarray(W_fcn, dtype=np.float32)
    a = np.asarray(a, dtype=np.float32)

    B = x.shape[0]
    bc = B // n_cores
    a_x, a_n = a[:H, 0], a[H:, 0]
    w6x_np, split_x = _score_weights(W_att, a_x)
    w6n_np, split_n = _score_weights(W_att, a_n)

    nc = _build_program(bc, split_n, split_x, n_cores)

    def cful(split, rep=1):
        v = np.concatenate([np.ones(split), -np.ones(HW6 - split)])
        v = np.tile(v, rep)
        return np.repeat(v[None, :].astype(np.float16), 128, axis=0)

    p = np.arange(128)
    psel4_np = np.equal.outer(p // 32, np.arange(4)).astype(np.float32)
    bsel4_np = np.equal.outer(np.arange(4), p // 32).astype(np.float32)
    bselx_np = np.equal.outer(p % 4, p // 32).astype(np.float32)
    selm_np = np.equal.outer(p // 4, np.arange(NB)).astype(np.float32)
    mask4_np = np.equal.outer(p // 32, np.arange(4)).astype(np.float16)

    shared = {
        "w6n": w6n_np.astype(np.float16), "w6x": w6x_np.astype(np.float16),
        "cfn": cful(split_n), "cfx": cful(split_x),
        "wfcx": W_fcx.astype(np.float16), "wfcn": W_fcn.astype(np.float16),
        "psel4": psel4_np, "bsel4": bsel4_np, "bselx": bselx_np,
        "selm": selm_np, "mask4": mask4_np,
    }

    rows_c = bc * NB
    tiles_c = rows_c // 128
    in_maps = []
    for c in range(n_cores):
        sl = neibs[c * rows_c:(c + 1) * rows_c]
        neT_np = np.ascontiguousarray(sl.T).astype(np.float16)
        neN_np = np.ascontiguousarray(
            sl.reshape(tiles_c, 128, D).transpose(1, 0, 2).reshape(128, rows_c)
        ).astype(np.float16)
        xT_np = np.ascontiguousarray(x[c * bc:(c + 1) * bc].T).astype(np.float16)
        in_maps.append({
            "neT": neT_np, "neN": neN_np, "xT": xT_np, **shared,
        })
    res = run_bass_kernel_spmd(nc, in_maps, core_ids=list(range(n_cores)),
                               **TRACE_OPTS)
    LAST_RESULT[0] = res
    return np.concatenate([res.results[c]["out"] for c in range(n_cores)], axis=0)


# revision 23
# speedup vs baseline: 2.0935x; 1.3390x over previous
"""AttentionAggregator Trainium2 kernel (8-core SPMD, data-parallel over nodes).

Math (per node b with neighbors n):
  x_att   = lrelu_.01(x @ W_att);  neib_att = lrelu_.01(neibs @ W_att)
  e[b,n]  = lrelu_.2(x_att[b]@a_x + neib_att[b,n]@a_n)
  att     = softmax_n(e)
  agg[b]  = sum_n att[b,n] * neibs[b,n]
  out     = relu([x@W_fcx, agg@W_fcn])

v3 design (vs the PE-transpose baseline):
  - Host pre-builds TWO fp16 layouts of neibs per core: neT (transposed
    [D, rows] — feeds score matmuls directly, no PE transposes) and neN
    (rows-grouped-by-128 natural [p, (tile, d)] — feeds the attention-apply
    matmuls).  Both DMA at 8KB/partition-line per block.
  - x is pre-transposed on host (xT [D, B]) and persists in SBUF: serves as
    lhsT for both the x-score matmul and the x@W_fcx output matmul.
  - Scores: 258-col relu-pair decomposition of a.lrelu(W z) (exact), drained
    by DVE scalar_tensor_tensor(max0, *cful, accum) / ACT-relu+GPSIMD-reduce.
  - Softmax normalization without cross-partition shuffles: per-node sums,
    reciprocal-broadcast and x-score broadcast are done with tiny constant
    selector matmuls on the PE (psel4/bsel4/bselx).
"""
import warnings
warnings.filterwarnings("ignore")
import numpy as np
from contextlib import ExitStack

import concourse.bass as bass
import concourse.tile as tile
from concourse import bacc, mybir
from concourse.bass_utils import run_bass_kernel_spmd

F32 = mybir.dt.float32
F16 = mybir.dt.float16
AF = mybir.ActivationFunctionType
ALU = mybir.AluOpType
AX = mybir.AxisListType

N_CORES = 8
B_FULL, NB, D, H, O = 20000, 32, 128, 256, 128
HW6 = 2 * H // 2 + 2  # 258 score columns

# Test-harness knobs (ignored by the grading harness, which calls kernel()
# directly): set TRACE_OPTS["trace"]=True to capture an NTFF profile; the
# BassKernelResults of the last run lands in LAST_RESULT[0].
TRACE_OPTS = {}
LAST_RESULT = [None]

# Drain scheduling: score PSUM tiles are consumed via an ACT relu pass over a
# 4-tile PSUM quad into SBUF fp16, then per-quad GROUPED tensor_reduce ops on
# DVE ([128,4,seg]->[128,4], one instruction per sign-segment, no accumulator
# read).  Odd tail tiles fall back to a direct DVE scalar_tensor_tensor drain.
N_QUADS = 8  # ACT-relu quads per 32-tile block (4 tiles each); rest DVE-direct


def _score_weights(W_att: np.ndarray, a_half: np.ndarray):
    """Build the 258-column relu-pair score weight matrix. Returns (W6, split)."""
    pos = np.where(a_half >= 0)[0]
    neg = np.where(a_half < 0)[0]
    Wabs = W_att * np.abs(a_half)[None, :]
    w_d = (W_att @ a_half).astype(np.float64)
    seg1 = np.concatenate([0.99 * Wabs[:, pos], 0.01 * w_d[:, None]], axis=1)
    seg2 = np.concatenate([0.99 * Wabs[:, neg], -0.01 * w_d[:, None]], axis=1)
    W6 = np.concatenate([seg1, seg2], axis=1).astype(np.float32)
    return W6, seg1.shape[1]


def _blocks(bc):
    out = []
    o = 0
    while o < bc:
        f = min(128, bc - o)
        assert f * NB % 128 == 0
        out.append((o, f))
        o += f
    return out


_PROG_CACHE = {}


def _build_program(bc, split_n, split_x, n_cores=N_CORES):
    key = (bc, split_n, split_x, n_cores, N_QUADS)
    if key in _PROG_CACHE:
        return _PROG_CACHE[key]

    nc = bacc.Bacc("TRN2", target_bir_lowering=False, debug=False,
                   num_devices=n_cores)

    R = bc * NB  # neighbor rows per core
    neT_d = nc.dram_tensor("neT", [D, R], F16, kind="ExternalInput").ap()
    neN_d = nc.dram_tensor("neN", [128, R], F16, kind="ExternalInput").ap()
    xT_d = nc.dram_tensor("xT", [D, bc], F16, kind="ExternalInput").ap()
    w6n_d = nc.dram_tensor("w6n", [D, HW6], F16, kind="ExternalInput").ap()
    w6x_d = nc.dram_tensor("w6x", [D, HW6], F16, kind="ExternalInput").ap()
    cfn_d = nc.dram_tensor("cfn", [128, HW6], F16, kind="ExternalInput").ap()
    cfx_d = nc.dram_tensor("cfx", [128, HW6], F16, kind="ExternalInput").ap()
    wfcx_d = nc.dram_tensor("wfcx", [D, O], F16, kind="ExternalInput").ap()
    wfcn_d = nc.dram_tensor("wfcn", [D, O], F16, kind="ExternalInput").ap()
    psel4_d = nc.dram_tensor("psel4", [128, 4], F32, kind="ExternalInput").ap()
    bsel4_d = nc.dram_tensor("bsel4", [4, 128], F32, kind="ExternalInput").ap()
    bselx_d = nc.dram_tensor("bselx", [128, 128], F32, kind="ExternalInput").ap()
    selm_d = nc.dram_tensor("selm", [128, NB], F32, kind="ExternalInput").ap()
    mask4_d = nc.dram_tensor("mask4", [128, 4], F16, kind="ExternalInput").ap()
    out_d = nc.dram_tensor("out", [bc, 2 * O], F32, kind="ExternalOutput").ap()

    with tile.TileContext(nc) as tc, ExitStack() as ctx:
        consts = ctx.enter_context(tc.tile_pool(name="consts", bufs=1))
        netp = ctx.enter_context(tc.tile_pool(name="netp", bufs=2))
        nenp = ctx.enter_context(tc.tile_pool(name="nenp", bufs=2))
        scrp = ctx.enter_context(tc.tile_pool(name="scrp", bufs=3))
        smallp = ctx.enter_context(tc.tile_pool(name="smallp", bufs=2))
        awp = ctx.enter_context(tc.tile_pool(name="awp", bufs=2))
        outp = ctx.enter_context(tc.tile_pool(name="outp", bufs=2))
        ps_sc = ctx.enter_context(tc.tile_pool(name="ps_sc", bufs=1, space="PSUM"))
        ps_pair = ctx.enter_context(tc.tile_pool(name="ps_pair", bufs=2, space="PSUM"))
        ps_misc = ctx.enter_context(tc.tile_pool(name="ps_misc", bufs=1, space="PSUM"))
        ps_agg = ctx.enter_context(tc.tile_pool(name="ps_agg", bufs=1, space="PSUM"))
        ps_fc = ctx.enter_context(tc.tile_pool(name="ps_fc", bufs=1, space="PSUM"))

        w6n = consts.tile([D, HW6], F16)
        w6x = consts.tile([D, HW6], F16)
        cfn = consts.tile([128, HW6], F16)
        cfx = consts.tile([128, HW6], F16)
        wfcx = consts.tile([D, O], F16)
        wfcn = consts.tile([D, O], F16)
        psel4 = consts.tile([128, 4], F32)
        bsel4 = consts.tile([4, 128], F32)
        bselx = consts.tile([128, 128], F32)
        selm = consts.tile([128, NB], F32)
        mask4 = consts.tile([128, 4], F16)
        xt_all = consts.tile([D, bc], F16)
        for t, d in [(w6n, w6n_d), (w6x, w6x_d), (cfn, cfn_d), (cfx, cfx_d),
                     (wfcx, wfcx_d), (wfcn, wfcn_d), (psel4, psel4_d),
                     (bsel4, bsel4_d), (bselx, bselx_d), (selm, selm_d),
                     (mask4, mask4_d), (xt_all, xT_d)]:
            nc.sync.dma_start(t[:], d)

        def phaseA(boff, F):
            """DMA loads, score matmuls + drains, x-side score."""
            T = F * NB // 128
            rbase = boff * NB

            netb = netp.tile([128, 32 * D], F16, tag="net")
            nc.sync.dma_start(netb[:, :T * D], neT_d[:, rbase:rbase + 128 * T])
            nenb = nenp.tile([128, 32 * D], F16, tag="nen")
            nc.sync.dma_start(nenb[:, :T * D], neN_d[:, rbase:rbase + 128 * T])

            scolP = smallp.tile([128, NB], F32, tag="scolP")
            scolN = smallp.tile([128, NB], F32, tag="scolN")
            nc.gpsimd.memset(scolN[:, :T], 0.0)

            # x-side score (drain on DVE)
            xs_ps = ps_sc.tile([128, HW6], F32, tag="sc")
            nc.tensor.matmul(xs_ps[:F, :], xt_all[:, boff:boff + F], w6x[:],
                             start=True, stop=True)
            xscr = scrp.tile([128, HW6], F16, tag="xscr")
            sx = smallp.tile([128, 1], F32, tag="sx")
            nc.vector.scalar_tensor_tensor(
                xscr[:F, :], xs_ps[:F, :], 0.0, cfx[:F, :],
                op0=ALU.max, op1=ALU.mult, accum_out=sx[:F, :])

            # neighbor score tiles: ACT-relu pairs + grouped DVE tensor_reduce
            # per sign-segment; odd tail tiles drain directly on DVE
            s1 = split_n
            t = 0
            while t < T:
                if t + 2 <= T:
                    qp = ps_pair.tile([128, 1024], F32, tag="qp")
                    qv = qp[:].rearrange("p (g c) -> p g c", c=512)
                    for g in range(2):
                        nc.tensor.matmul(qv[:, g, :HW6],
                                         netb[:, (t + g) * D:(t + g + 1) * D],
                                         w6n[:], start=True, stop=True)
                    scr2 = scrp.tile([128, 2 * HW6], F16, tag="scr2")
                    scr2v = scr2[:].rearrange("p (g c) -> p g c", c=HW6)
                    nc.scalar.activation(scr2v, qv[:, :, :HW6], AF.Relu)
                    nc.vector.tensor_reduce(
                        scolP[:, t:t + 2], scr2v[:, :, :s1],
                        axis=AX.X, op=ALU.add)
                    nc.vector.tensor_reduce(
                        scolN[:, t:t + 2], scr2v[:, :, s1:HW6],
                        axis=AX.X, op=ALU.add)
                    t += 2
                else:
                    s_ps = ps_sc.tile([128, HW6], F32, tag="sc")
                    nc.tensor.matmul(s_ps[:], netb[:, t * D:(t + 1) * D],
                                     w6n[:], start=True, stop=True)
                    scr = scrp.tile([128, HW6], F16, tag="scr")
                    nc.vector.scalar_tensor_tensor(
                        scr[:], s_ps[:], 0.0, cfn[:],
                        op0=ALU.max, op1=ALU.mult,
                        accum_out=scolP[:, t:t + 1])
                    t += 1

            return dict(nenb=nenb, T=T, F=F, boff=boff, scolP=scolP,
                        scolN=scolN, sx=sx)

        def phaseB(st):
            """Softmax (unnormalized exp + reciprocal-Z) and attention weights."""
            T, F = st["T"], st["F"]
            scolP, scolN, sx = st["scolP"], st["scolN"], st["sx"]

            # sxs[p,t] = sx[4t + p//32] via selector matmul
            Rm = smallp.tile([128, NB], F32, tag="Rm")
            nc.gpsimd.tensor_tensor(
                Rm[:, :T], sx[:, 0:1].broadcast_to([128, T]), selm[:, :T],
                op=ALU.mult)
            sxs_ps = ps_misc.tile([128, NB], F32, tag="misc")
            nc.tensor.matmul(sxs_ps[:, :T], bselx[:], Rm[:, :T],
                             start=True, stop=True)

            z0 = smallp.tile([128, NB], F32, tag="z0")
            nc.gpsimd.tensor_tensor(z0[:, :T], scolP[:, :T], scolN[:, :T],
                                    op=ALU.subtract)
            z = smallp.tile([128, NB], F32, tag="z")
            nc.vector.tensor_tensor(z[:, :T], z0[:, :T], sxs_ps[:, :T],
                                    op=ALU.add)
            zl = smallp.tile([128, NB], F32, tag="zl")
            nc.vector.scalar_tensor_tensor(zl[:, :T], z[:, :T], 0.2, z[:, :T],
                                           op0=ALU.mult, op1=ALU.max)
            ew = smallp.tile([128, NB], F32, tag="ew")
            nc.scalar.activation(ew[:, :T], zl[:, :T], AF.Exp)

            # Z per node, reciprocal, broadcast back to [p, t]
            zt_ps = ps_misc.tile([128, NB], F32, tag="misc")
            nc.tensor.matmul(zt_ps[:4, :T], psel4[:], ew[:, :T],
                             start=True, stop=True)
            rz4 = smallp.tile([4, NB], F32, tag="rz4")
            nc.vector.reciprocal(rz4[:4, :T], zt_ps[:4, :T])
            rzf_ps = ps_misc.tile([128, NB], F32, tag="misc")
            nc.tensor.matmul(rzf_ps[:, :T], bsel4[:4, :], rz4[:4, :T],
                             start=True, stop=True)
            ewn = smallp.tile([128, NB], F32, tag="ewn")
            nc.vector.tensor_tensor(ewn[:, :T], ew[:, :T], rzf_ps[:, :T],
                                    op=ALU.mult)

            aw = awp.tile([128, 128], F16, tag="aw")
            nc.gpsimd.tensor_tensor(
                aw[:].rearrange("p (t j) -> p t j", j=4)[:, :T, :],
                ewn[:, :T].unsqueeze(2).broadcast_to([128, T, 4]),
                mask4[:].unsqueeze(1).broadcast_to([128, T, 4]),
                op=ALU.mult)
            st["aw"] = aw

        def phaseC(st):
            """Attention apply (agg), output matmuls, relu, store."""
            nenb, aw = st["nenb"], st["aw"]
            T, F, boff = st["T"], st["F"], st["boff"]
            nen_v = nenb[:].rearrange("p (t d) -> p t d", d=D)
            aw_v = aw[:].rearrange("p (t j) -> p t j", j=4)
            agg_ps = ps_agg.tile([128, 128], F32, tag="agg")
            for t in range(T):
                nc.tensor.matmul(agg_ps[:, 4 * t:4 * (t + 1)], nen_v[:, t, :],
                                 aw_v[:, t, :], start=True, stop=True)
            aggt = awp.tile([D, 128], F16, tag="aggt")
            nc.vector.tensor_copy(aggt[:, :F], agg_ps[:, :F])

            fc_ps = ps_fc.tile([128, 2 * O], F32, tag="fc")
            nc.tensor.matmul(fc_ps[:F, 0:O], xt_all[:, boff:boff + F], wfcx[:],
                             start=True, stop=True)
            nc.tensor.matmul(fc_ps[:F, O:2 * O], aggt[:, :F], wfcn[:],
                             start=True, stop=True)
            out_sb = outp.tile([128, 2 * O], F32, tag="out")
            nc.scalar.activation(out_sb[:F, :], fc_ps[:F, :], AF.Relu)
            nc.sync.dma_start(out_d[boff:boff + F, :], out_sb[:F, :])

        prev = None
        for (boff, F) in _blocks(bc):
            st = phaseA(boff, F)
            if prev is not None:
                phaseC(prev)
            phaseB(st)
            prev = st
        phaseC(prev)

    nc.compile()
    _PROG_CACHE[key] = nc
    return nc


def kernel(x, neibs, W_att, W_fcx, W_fcn, a, n_cores=N_CORES):
    x = np.asarray(x, dtype=np.float32)
    neibs = np.asarray(neibs, dtype=np.float32)
    W_att = np.asarray(W_att, dtype=np.float32)
    W_fcx = np.asarray(W_fcx, dtype=np.float32)
    W_fcn = np.asarray(W_fcn, dtype=np.float32)
    a = np.asarray(a, dtype=np.float32)

    B = x.shape[0]
    bc = B // n_cores
    a_x, a_n = a[:H, 0], a[H:, 0]
    w6x_np, split_x = _score_weights(W_att, a_x)
    w6n_np, split_n = _score_weights(W_att, a_n)

    nc = _build_program(bc, split_n, split_x, n_cores)

    def cful(split, rep=1):
        v = np.concatenate([np.ones(split), -np.ones(HW6 - split)])
        v = np.tile(v, rep)
        return np.repeat(v[None, :].astype(np.float16), 128, axis=0)

    p = np.arange(128)
    psel4_np = np.equal.outer(p // 32, np.arange(4)).astype(np.float32)
    bsel4_np = np.equal.outer(np.arange(4), p // 32).astype(np.float32)
    bselx_np = np.equal.outer(p % 4, p // 32).astype(np.float32)
    selm_np = np.equal.outer(p // 4, np.arange(NB)).astype(np.float32)
    mask4_np = np.equal.outer(p // 32, np.arange(4)).astype(np.float16)

    shared = {
        "w6n": w6n_np.astype(np.float16), "w6x": w6x_np.astype(np.float16),
        "cfn": cful(split_n), "cfx": cful(split_x),
        "wfcx": W_fcx.astype(np.float16), "wfcn": W_fcn.astype(np.float16),
        "psel4": psel4_np, "bsel4": bsel4_np, "bselx": bselx_np,
        "selm": selm_np, "mask4": mask4_np,
    }

    rows_c = bc * NB
    tiles_c = rows_c // 128
    in_maps = []
    for c in range(n_cores):
        sl = neibs[c * rows_c:(c + 1) * rows_c]
        neT_np = np.ascontiguousarray(sl.T).astype(np.float16)
        neN_np = np.ascontiguousarray(
            sl.reshape(tiles_c, 128, D).transpose(1, 0, 2).reshape(128, rows_c)
        ).astype(np.float16)
        xT_np = np.ascontiguousarray(x[c * bc:(c + 1) * bc].T).astype(np.float16)
        in_maps.append({
            "neT": neT_np, "neN": neN_np, "xT": xT_np, **shared,
        })
    res = run_bass_kernel_spmd(nc, in_maps, core_ids=list(range(n_cores)),
                               **TRACE_OPTS)
    LAST_RESULT[0] = res
    return np.concatenate([res.results[c]["out"] for c in range(n_cores)], axis=0)


# revision 24
# speedup vs baseline: 2.1766x; 1.0397x over previous
"""AttentionAggregator Trainium2 kernel (8-core SPMD, data-parallel over nodes).

Math (per node b with neighbors n):
  x_att   = lrelu_.01(x @ W_att);  neib_att = lrelu_.01(neibs @ W_att)
  e[b,n]  = lrelu_.2(x_att[b]@a_x + neib_att[b,n]@a_n)
  att     = softmax_n(e)
  agg[b]  = sum_n att[b,n] * neibs[b,n]
  out     = relu([x@W_fcx, agg@W_fcn])

v3 design (vs the PE-transpose baseline):
  - Host pre-builds TWO fp16 layouts of neibs per core: neT (transposed
    [D, rows] — feeds score matmuls directly, no PE transposes) and neN
    (rows-grouped-by-128 natural [p, (tile, d)] — feeds the attention-apply
    matmuls).  Both DMA at 8KB/partition-line per block.
  - x is pre-transposed on host (xT [D, B]) and persists in SBUF: serves as
    lhsT for both the x-score matmul and the x@W_fcx output matmul.
  - Scores: 258-col relu-pair decomposition of a.lrelu(W z) (exact), drained
    by DVE scalar_tensor_tensor(max0, *cful, accum) / ACT-relu+GPSIMD-reduce.
  - Softmax normalization without cross-partition shuffles: per-node sums,
    reciprocal-broadcast and x-score broadcast are done with tiny constant
    selector matmuls on the PE (psel4/bsel4/bselx).
"""
import warnings
warnings.filterwarnings("ignore")
import numpy as np
from contextlib import ExitStack

import concourse.bass as bass
import concourse.tile as tile
from concourse import bacc, mybir
from concourse.bass_utils import run_bass_kernel_spmd

F32 = mybir.dt.float32
F16 = mybir.dt.float16
AF = mybir.ActivationFunctionType
ALU = mybir.AluOpType
AX = mybir.AxisListType

N_CORES = 8
B_FULL, NB, D, H, O = 20000, 32, 128, 256, 128
HW6 = 2 * H // 2 + 2  # 258 score columns

# Test-harness knobs (ignored by the grading harness, which calls kernel()
# directly): set TRACE_OPTS["trace"]=True to capture an NTFF profile; the
# BassKernelResults of the last run lands in LAST_RESULT[0].
TRACE_OPTS = {}
LAST_RESULT = [None]

# Drain scheduling: score PSUM tiles are consumed via an ACT relu pass over a
# 4-tile PSUM quad into SBUF fp16, then per-quad GROUPED tensor_reduce ops on
# DVE ([128,4,seg]->[128,4], one instruction per sign-segment, no accumulator
# read).  Odd tail tiles fall back to a direct DVE scalar_tensor_tensor drain.
N_QUADS = 8  # ACT-relu quads per 32-tile block (4 tiles each); rest DVE-direct


def _score_weights(W_att: np.ndarray, a_half: np.ndarray):
    """Build the 258-column relu-pair score weight matrix. Returns (W6, split)."""
    pos = np.where(a_half >= 0)[0]
    neg = np.where(a_half < 0)[0]
    Wabs = W_att * np.abs(a_half)[None, :]
    w_d = (W_att @ a_half).astype(np.float64)
    seg1 = np.concatenate([0.99 * Wabs[:, pos], 0.01 * w_d[:, None]], axis=1)
    seg2 = np.concatenate([0.99 * Wabs[:, neg], -0.01 * w_d[:, None]], axis=1)
    W6 = np.concatenate([seg1, seg2], axis=1).astype(np.float32)
    return W6, seg1.shape[1]


def _blocks(bc):
    out = []
    o = 0
    while o < bc:
        f = min(128, bc - o)
        assert f * NB % 128 == 0
        out.append((o, f))
        o += f
    return out


_PROG_CACHE = {}


def _build_program(bc, split_n, split_x, n_cores=N_CORES):
    key = (bc, split_n, split_x, n_cores, N_QUADS)
    if key in _PROG_CACHE:
        return _PROG_CACHE[key]

    nc = bacc.Bacc("TRN2", target_bir_lowering=False, debug=False,
                   num_devices=n_cores)

    R = bc * NB  # neighbor rows per core
    neT_d = nc.dram_tensor("neT", [D, R], F16, kind="ExternalInput").ap()
    neN_d = nc.dram_tensor("neN", [128, R], F16, kind="ExternalInput").ap()
    xT_d = nc.dram_tensor("xT", [D, bc], F16, kind="ExternalInput").ap()
    w6n_d = nc.dram_tensor("w6n", [D, HW6], F16, kind="ExternalInput").ap()
    w6x_d = nc.dram_tensor("w6x", [D, HW6], F16, kind="ExternalInput").ap()
    cfn_d = nc.dram_tensor("cfn", [128, HW6], F16, kind="ExternalInput").ap()
    cfx_d = nc.dram_tensor("cfx", [128, HW6], F16, kind="ExternalInput").ap()
    wfcx_d = nc.dram_tensor("wfcx", [D, O], F16, kind="ExternalInput").ap()
    wfcn_d = nc.dram_tensor("wfcn", [D, O], F16, kind="ExternalInput").ap()
    psel4_d = nc.dram_tensor("psel4", [128, 4], F32, kind="ExternalInput").ap()
    bsel4_d = nc.dram_tensor("bsel4", [4, 128], F32, kind="ExternalInput").ap()
    bselx_d = nc.dram_tensor("bselx", [128, 128], F32, kind="ExternalInput").ap()
    selm_d = nc.dram_tensor("selm", [128, NB], F32, kind="ExternalInput").ap()
    mask4_d = nc.dram_tensor("mask4", [128, 4], F16, kind="ExternalInput").ap()
    out_d = nc.dram_tensor("out", [bc, 2 * O], F32, kind="ExternalOutput").ap()

    with tile.TileContext(nc) as tc, ExitStack() as ctx:
        consts = ctx.enter_context(tc.tile_pool(name="consts", bufs=1))
        netp = ctx.enter_context(tc.tile_pool(name="netp", bufs=2))
        nenp = ctx.enter_context(tc.tile_pool(name="nenp", bufs=2))
        scrp = ctx.enter_context(tc.tile_pool(name="scrp", bufs=3))
        smallp = ctx.enter_context(tc.tile_pool(name="smallp", bufs=2))
        awp = ctx.enter_context(tc.tile_pool(name="awp", bufs=2))
        outp = ctx.enter_context(tc.tile_pool(name="outp", bufs=2))
        ps_sc = ctx.enter_context(tc.tile_pool(name="ps_sc", bufs=1, space="PSUM"))
        ps_pair = ctx.enter_context(tc.tile_pool(name="ps_pair", bufs=2, space="PSUM"))
        ps_misc = ctx.enter_context(tc.tile_pool(name="ps_misc", bufs=1, space="PSUM"))
        ps_agg = ctx.enter_context(tc.tile_pool(name="ps_agg", bufs=1, space="PSUM"))
        ps_fc = ctx.enter_context(tc.tile_pool(name="ps_fc", bufs=1, space="PSUM"))

        w6n = consts.tile([D, HW6], F16)
        w6x = consts.tile([D, HW6], F16)
        cfn = consts.tile([128, HW6], F16)
        cfx = consts.tile([128, HW6], F16)
        wfcx = consts.tile([D, O], F16)
        wfcn = consts.tile([D, O], F16)
        psel4 = consts.tile([128, 4], F32)
        bsel4 = consts.tile([4, 128], F32)
        bselx = consts.tile([128, 128], F32)
        selm = consts.tile([128, NB], F32)
        mask4 = consts.tile([128, 4], F16)
        xt_all = consts.tile([D, bc], F16)
        for t, d in [(w6n, w6n_d), (w6x, w6x_d), (cfn, cfn_d), (cfx, cfx_d),
                     (wfcx, wfcx_d), (wfcn, wfcn_d), (psel4, psel4_d),
                     (bsel4, bsel4_d), (bselx, bselx_d), (selm, selm_d),
                     (mask4, mask4_d), (xt_all, xT_d)]:
            nc.sync.dma_start(t[:], d)

        def phaseA(boff, F):
            """DMA loads, score matmuls + drains, x-side score."""
            T = F * NB // 128
            rbase = boff * NB

            netb = netp.tile([128, 32 * D], F16, tag="net")
            nc.sync.dma_start(netb[:, :T * D], neT_d[:, rbase:rbase + 128 * T])
            nenb = nenp.tile([128, 32 * D], F16, tag="nen")
            nc.sync.dma_start(nenb[:, :T * D], neN_d[:, rbase:rbase + 128 * T])

            scolP = smallp.tile([128, NB], F32, tag="scolP")
            scolN = smallp.tile([128, NB], F32, tag="scolN")
            nc.gpsimd.memset(scolN[:, :T], 0.0)

            # x-side score (drain on DVE)
            xs_ps = ps_sc.tile([128, HW6], F32, tag="sc")
            nc.tensor.matmul(xs_ps[:F, :], xt_all[:, boff:boff + F], w6x[:],
                             start=True, stop=True)
            xscr = scrp.tile([128, HW6], F16, tag="xscr")
            sx = smallp.tile([128, 1], F32, tag="sx")
            nc.vector.scalar_tensor_tensor(
                xscr[:F, :], xs_ps[:F, :], 0.0, cfx[:F, :],
                op0=ALU.max, op1=ALU.mult, accum_out=sx[:F, :])

            # neighbor score tiles: PSUM pairs relu'd by ACT into halves of a
            # 4-tile fp16 buffer; DVE then does one grouped tensor_reduce per
            # sign-segment covering 4 tiles.  Odd tail tile drains direct.
            s1 = split_n
            t = 0
            pend = None

            def reduce_group(scr4, t0, G):
                v = scr4[:].rearrange("p (g c) -> p g c", c=HW6)
                nc.vector.tensor_reduce(
                    scolP[:, t0:t0 + G], v[:, :G, :s1], axis=AX.X, op=ALU.add)
                nc.vector.tensor_reduce(
                    scolN[:, t0:t0 + G], v[:, :G, s1:HW6], axis=AX.X,
                    op=ALU.add)

            while t < T:
                if t + 2 <= T:
                    qp = ps_pair.tile([128, 1024], F32, tag="qp")
                    qv = qp[:].rearrange("p (g c) -> p g c", c=512)
                    for g in range(2):
                        nc.tensor.matmul(qv[:, g, :HW6],
                                         netb[:, (t + g) * D:(t + g + 1) * D],
                                         w6n[:], start=True, stop=True)
                    if pend is None:
                        scr4 = scrp.tile([128, 4 * HW6], F16, tag="scr4")
                        half = scr4[:, :2 * HW6]
                        pend = (scr4, t)
                    else:
                        scr4, t0 = pend
                        half = scr4[:, 2 * HW6:]
                    nc.scalar.activation(
                        half.rearrange("p (g c) -> p g c", c=HW6),
                        qv[:, :, :HW6], AF.Relu)
                    if pend[1] != t:
                        reduce_group(scr4, t0, 4)
                        pend = None
                    t += 2
                else:
                    s_ps = ps_sc.tile([128, HW6], F32, tag="sc")
                    nc.tensor.matmul(s_ps[:], netb[:, t * D:(t + 1) * D],
                                     w6n[:], start=True, stop=True)
                    scr = scrp.tile([128, HW6], F16, tag="scr")
                    nc.vector.scalar_tensor_tensor(
                        scr[:], s_ps[:], 0.0, cfn[:],
                        op0=ALU.max, op1=ALU.mult,
                        accum_out=scolP[:, t:t + 1])
                    t += 1
            if pend is not None:
                reduce_group(pend[0], pend[1], 2)

            return dict(nenb=nenb, T=T, F=F, boff=boff, scolP=scolP,
                        scolN=scolN, sx=sx)

        def phaseB(st):
            """Softmax (unnormalized exp + reciprocal-Z) and attention weights."""
            T, F = st["T"], st["F"]
            scolP, scolN, sx = st["scolP"], st["scolN"], st["sx"]

            # sxs[p,t] = sx[4t + p//32] via selector matmul
            Rm = smallp.tile([128, NB], F32, tag="Rm")
            nc.gpsimd.tensor_tensor(
                Rm[:, :T], sx[:, 0:1].broadcast_to([128, T]), selm[:, :T],
                op=ALU.mult)
            sxs_ps = ps_misc.tile([128, NB], F32, tag="misc")
            nc.tensor.matmul(sxs_ps[:, :T], bselx[:], Rm[:, :T],
                             start=True, stop=True)

            z0 = smallp.tile([128, NB], F32, tag="z0")
            nc.gpsimd.tensor_tensor(z0[:, :T], scolP[:, :T], scolN[:, :T],
                                    op=ALU.subtract)
            z = smallp.tile([128, NB], F32, tag="z")
            nc.vector.tensor_tensor(z[:, :T], z0[:, :T], sxs_ps[:, :T],
                                    op=ALU.add)
            zl = smallp.tile([128, NB], F32, tag="zl")
            nc.vector.scalar_tensor_tensor(zl[:, :T], z[:, :T], 0.2, z[:, :T],
                                           op0=ALU.mult, op1=ALU.max)
            ew = smallp.tile([128, NB], F32, tag="ew")
            nc.scalar.activation(ew[:, :T], zl[:, :T], AF.Exp)

            # Z per node, reciprocal, broadcast back to [p, t]
            zt_ps = ps_misc.tile([128, NB], F32, tag="misc")
            nc.tensor.matmul(zt_ps[:4, :T], psel4[:], ew[:, :T],
                             start=True, stop=True)
            rz4 = smallp.tile([4, NB], F32, tag="rz4")
            nc.vector.reciprocal(rz4[:4, :T], zt_ps[:4, :T])
            rzf_ps = ps_misc.tile([128, NB], F32, tag="misc")
            nc.tensor.matmul(rzf_ps[:, :T], bsel4[:4, :], rz4[:4, :T],
                             start=True, stop=True)
            ewn = smallp.tile([128, NB], F32, tag="ewn")
            nc.vector.tensor_tensor(ewn[:, :T], ew[:, :T], rzf_ps[:, :T],
                                    op=ALU.mult)

            aw = awp.tile([128, 128], F16, tag="aw")
            nc.gpsimd.tensor_tensor(
                aw[:].rearrange("p (t j) -> p t j", j=4)[:, :T, :],
                ewn[:, :T].unsqueeze(2).broadcast_to([128, T, 4]),
                mask4[:].unsqueeze(1).broadcast_to([128, T, 4]),
                op=ALU.mult)
            st["aw"] = aw

        def phaseC(st):
            """Attention apply (agg), output matmuls, relu, store."""
            nenb, aw = st["nenb"], st["aw"]
            T, F, boff = st["T"], st["F"], st["boff"]
            nen_v = nenb[:].rearrange("p (t d) -> p t d", d=D)
            aw_v = aw[:].rearrange("p (t j) -> p t j", j=4)
            agg_ps = ps_agg.tile([128, 128], F32, tag="agg")
            for t in range(T):
                nc.tensor.matmul(agg_ps[:, 4 * t:4 * (t + 1)], nen_v[:, t, :],
                                 aw_v[:, t, :], start=True, stop=True)
            aggt = awp.tile([D, 128], F16, tag="aggt")
            nc.vector.tensor_copy(aggt[:, :F], agg_ps[:, :F])

            fc_ps = ps_fc.tile([128, 2 * O], F32, tag="fc")
            nc.tensor.matmul(fc_ps[:F, 0:O], xt_all[:, boff:boff + F], wfcx[:],
                             start=True, stop=True)
            nc.tensor.matmul(fc_ps[:F, O:2 * O], aggt[:, :F], wfcn[:],
                             start=True, stop=True)
            out_sb = outp.tile([128, 2 * O], F32, tag="out")
            nc.scalar.activation(out_sb[:F, :], fc_ps[:F, :], AF.Relu)
            nc.sync.dma_start(out_d[boff:boff + F, :], out_sb[:F, :])

        prev = None
        for (boff, F) in _blocks(bc):
            st = phaseA(boff, F)
            if prev is not None:
                phaseC(prev)
            phaseB(st)
            prev = st
        phaseC(prev)

    nc.compile()
    _PROG_CACHE[key] = nc
    return nc


def kernel(x, neibs, W_att, W_fcx, W_fcn, a, n_cores=N_CORES):
    x = np.asarray(x, dtype=np.float32)
    neibs = np.asarray(neibs, dtype=np.float32)
    W_att = np.asarray(W_att, dtype=np.float32)
    W_fcx = np.asarray(W_fcx, dtype=np.float32)
    W_fcn = np.asarray(W_fcn, dtype=np.float32)
    a = np.asarray(a, dtype=np.float32)

    B = x.shape[0]
    bc = B // n_cores
    a_x, a_n = a[:H, 0], a[H:, 0]
    w6x_np, split_x = _score_weights(W_att, a_x)
    w6n_np, split_n = _score_weights(W_att, a_n)

    nc = _build_program(bc, split_n, split_x, n_cores)

    def cful(split, rep=1):
        v = np.concatenate([np.ones(split), -np.ones(HW6 - split)])
        v = np.tile(v, rep)
        return np.repeat(v[None, :].astype(np.float16), 128, axis=0)

    p = np.arange(128)
    psel4_np = np.equal.outer(p // 32, np.arange(4)).astype(np.float32)
    bsel4_np = np.equal.outer(np.arange(4), p // 32).astype(np.float32)
    bselx_np = np.equal.outer(p % 4, p // 32).astype(np.float32)
    selm_np = np.equal.outer(p // 4, np.arange(NB)).astype(np.float32)
    mask4_np = np.equal.outer(p // 32, np.arange(4)).astype(np.float16)

    shared = {
        "w6n": w6n_np.astype(np.float16), "w6x": w6x_np.astype(np.float16),
        "cfn": cful(split_n), "cfx": cful(split_x),
        "wfcx": W_fcx.astype(np.float16), "wfcn": W_fcn.astype(np.float16),
        "psel4": psel4_np, "bsel4": bsel4_np, "bselx": bselx_np,
        "selm": selm_np, "mask4": mask4_np,
    }

    rows_c = bc * NB
    tiles_c = rows_c // 128
    in_maps = []
    for c in range(n_cores):
        sl = neibs[c * rows_c:(c + 1) * rows_c]
        neT_np = np.ascontiguousarray(sl.T).astype(np.float16)
        neN_np = np.ascontiguousarray(
            sl.reshape(tiles_c, 128, D).transpose(1, 0, 2).reshape(128, rows_c)
        ).astype(np.float16)
        xT_np = np.ascontiguousarray(x[c * bc:(c + 1) * bc].T).astype(np.float16)
        in_maps.append({
            "neT": neT_np, "neN": neN_np, "xT": xT_np, **shared,
        })
    res = run_bass_kernel_spmd(nc, in_maps, core_ids=list(range(n_cores)),
                               **TRACE_OPTS)
    LAST_RESULT[0] = res
    return np.concatenate([res.results[c]["out"] for c in range(n_cores)], axis=0)
